# revision 1
# baseline (speedup 1.0000x reference)
"""Causal self-attention kernel for TRN2 (8 NeuronCores, Bass/Tile).

Problem: B=8, T=1024, C=768, H=12, HD=64.
  qkv = x @ W_attn + b_attn ; causal softmax attention ; y = att_out @ W_proj + b_proj

Sharding: pure data-parallel over batch - core b computes batch element b.

v3 design (all matmuls bf16):
  - x transposed on HOST -> xT loaded directly (no PE transposes).
  - W_qk host-permuted pair-major (256-col blocks per head pair) so the
    first attention pair's weights arrive first; per-(hp,cc) DMAs let
    the qk chains stream right behind the DMA.
  - b_attn qk part host-packed as per-partition columns [128, 12].
  - Vp head copies: one strided DVE copy per (tp, vc) (6 heads at once).
  - Fine-grained interleave: qk/v/proj matmul chains are chopped into
    small units and pumped into the PE queue between attention jc
    iterations, so the PE never stalls waiting for exp (Scalar) and
    stays at the fast p-state.
  - proj contracts ATn[5] last so only the last norm gates the tail.
"""

import numpy as np

import concourse.bass as bass
import concourse.mybir as mybir
import concourse.tile as tile
from concourse import bacc
from concourse.bass_utils import run_bass_kernel_spmd

F32 = mybir.dt.float32
BF16 = mybir.dt.bfloat16
AF = mybir.ActivationFunctionType
ALU = mybir.AluOpType

T, C, H, HD = 1024, 768, 12, 64
NCORES = 8
CC = C // 128          # 6 contraction chunks
TP = T // 128          # 8 t-chunks of 128
TB = T // 512          # 2 t-blocks of 512
NP = 6                 # head pairs
SCALE = 1.0 / 8.0      # 1/sqrt(64)

_PROGRAM_CACHE = {}


def build_program():
    nc = bacc.Bacc("TRN2", target_bir_lowering=False, debug=False)

    xt_d = nc.dram_tensor("xT", [C, T], BF16, kind="ExternalInput").ap()
    wqk_d = nc.dram_tensor("wqk", [C, 2 * C], BF16, kind="ExternalInput").ap()
    wv_d = nc.dram_tensor("Wv", [C, C], BF16, kind="ExternalInput").ap()
    wp_d = nc.dram_tensor("W_proj", [C, C], BF16, kind="ExternalInput").ap()
    bqk_d = nc.dram_tensor("bqk", [128, 12], F32, kind="ExternalInput").ap()
    bv_d = nc.dram_tensor("bv", [1, C], BF16, kind="ExternalInput").ap()
    bp_d = nc.dram_tensor("b_proj", [1, C], BF16, kind="ExternalInput").ap()
    y_d = nc.dram_tensor("y", [T, C], F32, kind="ExternalOutput").ap()

    with tile.TileContext(nc) as tc:
        _emit(nc, tc, xt_d, wqk_d, wv_d, wp_d, bqk_d, bv_d, bp_d, y_d)
    nc.compile()
    return nc


def _emit(nc, tc, xt_d, wqk_d, wv_d, wp_d, bqk_d, bv_d, bp_d, y_d):
    from contextlib import ExitStack
    from collections import deque

    ctx = ExitStack()
    with ctx:
        const_pool = ctx.enter_context(tc.tile_pool(name="consts", bufs=1))
        # ps_work: [128,1024] ST tiles (2 banks x 2 bufs); ps_acc: 1-bank
        # accumulators (qk/v/y); po: PV accumulators (tag "ot").
        ps_work = ctx.enter_context(tc.tile_pool(name="ps_work", bufs=2, space="PSUM"))
        ps_acc = ctx.enter_context(tc.tile_pool(name="ps_acc", bufs=2, space="PSUM"))

        # ---- input DMAs, interleaved in consumption order ---------------
        in_pool = ctx.enter_context(tc.tile_pool(name="inputs", bufs=1, side="right"))
        xT, Wqk, Wv = [], [], []
        for cc in range(CC):
            xT.append(in_pool.tile([128, T], BF16, name=f"xT_{cc}", tag=f"xT{cc}"))
            Wqk.append(in_pool.tile([128, 2 * C], BF16, name=f"Wqk_{cc}", tag=f"Wqk{cc}"))
            Wv.append(in_pool.tile([128, C], BF16, name=f"Wv_{cc}", tag=f"Wv{cc}"))
        for cc in range(CC):
            nc.sync.dma_start(xT[cc][:], xt_d[cc * 128 : (cc + 1) * 128, :])
            nc.sync.dma_start(
                Wqk[cc][:, 0:256], wqk_d[cc * 128 : (cc + 1) * 128, 0:256]
            )
        for cc in range(CC):
            nc.sync.dma_start(
                Wqk[cc][:, 256:512], wqk_d[cc * 128 : (cc + 1) * 128, 256:512]
            )
        bqk = const_pool.tile([128, 12], F32, name="bqk")
        nc.sync.dma_start(bqk[:], bqk_d[:, :])
        for cc in range(CC):
            nc.sync.dma_start(Wv[cc][:], wv_d[cc * 128 : (cc + 1) * 128, :])
        bv_sb = const_pool.tile([1, C], BF16, name="bv_sb")
        nc.sync.dma_start(bv_sb[:], bv_d[:, :])
        bp_sb = const_pool.tile([1, C], BF16, name="bp_sb")
        nc.sync.dma_start(bp_sb[:], bp_d[:, :])
        for cc in range(CC):
            nc.sync.dma_start(
                Wqk[cc][:, 512:1536], wqk_d[cc * 128 : (cc + 1) * 128, 512:1536]
            )
        wp_pool = ctx.enter_context(tc.tile_pool(name="wp", bufs=1))
        Wp = []
        for cc in range(CC):
            w_t = wp_pool.tile([128, C], BF16, name=f"Wp_{cc}", tag=f"Wp{cc}")
            nc.sync.dma_start(w_t[:], wp_d[cc * 128 : (cc + 1) * 128, :])
            Wp.append(w_t)

        # ---- constants -------------------------------------------------
        tri_f32 = const_pool.tile([128, 128], F32, name="tri_f32")
        nc.gpsimd.memset(tri_f32[:], 1.0)
        nc.gpsimd.affine_select(
            out=tri_f32[:], in_=tri_f32[:], compare_op=ALU.is_ge, fill=0.0,
            base=0, pattern=[[1, 128]], channel_multiplier=-1,
        )
        tri = const_pool.tile([128, 128], BF16, name="tri")
        nc.vector.tensor_copy(tri[:], tri_f32[:])
        ones32 = const_pool.tile([128, 16], F32, name="ones32")
        nc.gpsimd.memset(ones32[:], 1.0)
        ones_row = const_pool.tile([1, 512], BF16, name="ones_row")
        nc.gpsimd.memset(ones_row[:], 1.0)

        expwarm = const_pool.tile([1, 1], F32, name="expwarm")
        nc.scalar.activation(expwarm[:], ones32[0:1, 0:1], AF.Exp)

        # ---- qk projection (bf16) --------------------------------------
        qkt_pool = ctx.enter_context(tc.tile_pool(name="qkt", bufs=1))
        qkT = {}
        for hp in range(NP):
            for qk in range(2):
                qkT[(hp, qk)] = qkt_pool.tile(
                    [128, T], BF16, name=f"qkT_{hp}_{qk}", tag=f"qkT{hp}{qk}"
                )

        def qk_chain_units(hp, qk, tb):
            """Two units: a [128,512] projection chain split in halves."""
            col = hp * 256 + qk * 128
            pq = ps_acc.tile([128, 512], F32, name=f"ps_qk_{hp}_{qk}_{tb}", tag="acc")

            def u1():
                for cc in range(3):
                    nc.tensor.matmul(
                        pq[:],
                        Wqk[cc][:, col : col + 128],
                        xT[cc][:, tb * 512 : (tb + 1) * 512],
                        start=(cc == 0),
                        stop=False,
                    )

            def u2():
                for cc in range(3, CC):
                    nc.tensor.matmul(
                        pq[:],
                        Wqk[cc][:, col : col + 128],
                        xT[cc][:, tb * 512 : (tb + 1) * 512],
                        start=False,
                        stop=(cc == CC - 1),
                    )
                nc.vector.tensor_scalar_add(
                    qkT[(hp, qk)][:, tb * 512 : (tb + 1) * 512],
                    pq[:],
                    bqk[:, hp * 2 + qk : hp * 2 + qk + 1],
                )

            return [u1, u2]

        # ---- V path ----------------------------------------------------
        vp_pool = ctx.enter_context(tc.tile_pool(name="vp", bufs=1))
        Vp = []
        for tp in range(TP):
            t_ = vp_pool.tile([128, H * 65], BF16, name=f"Vp_{tp}", tag=f"Vp{tp}")
            Vp.append(t_)
            nc.vector.tensor_copy(
                t_.rearrange("p (h e) -> p h e", e=65)[:, :, 64:65],
                ones32[:, 0:H].rearrange("p (h e) -> p h e", e=1),
            )

        def v_chain_units(tp, vc):
            pv = ps_acc.tile([128, 384], F32, name=f"ps_v_{vc}_{tp}", tag="acc")

            def u1():
                for cc in range(3):
                    nc.tensor.matmul(
                        pv[:],
                        xT[cc][:, tp * 128 : (tp + 1) * 128],
                        Wv[cc][:, vc * 384 : (vc + 1) * 384],
                        start=(cc == 0),
                        stop=False,
                    )

            def u2():
                for cc in range(3, CC):
                    nc.tensor.matmul(
                        pv[:],
                        xT[cc][:, tp * 128 : (tp + 1) * 128],
                        Wv[cc][:, vc * 384 : (vc + 1) * 384],
                        start=False,
                        stop=False,
                    )
                nc.tensor.matmul(
                    pv[:],
                    ones_row[:, 0:128],
                    bv_sb[:, vc * 384 : (vc + 1) * 384],
                    start=False,
                    stop=True,
                )
                nc.vector.tensor_copy(
                    Vp[tp].rearrange("p (h e) -> p h e", e=65)[
                        :, 6 * vc : 6 * vc + 6, 0:64
                    ],
                    pv[:].rearrange("p (h e) -> p h e", e=64),
                )

            return [u1, u2]

        # ---- attention -------------------------------------------------
        atn_pool = ctx.enter_context(tc.tile_pool(name="atn", bufs=1))
        ATn = []
        for cp in range(CC):
            t_ = atn_pool.tile([128, T], BF16, name=f"ATn_{cp}", tag=f"ATn{cp}")
            ATn.append(t_)

        est_pool = ctx.enter_context(tc.tile_pool(name="est", bufs=8))
        nrm_pool = ctx.enter_context(tc.tile_pool(name="nrm", bufs=4))
        y_pool = ctx.enter_context(tc.tile_pool(name="ysb", bufs=2))

        fillers = deque()

        def pump(n):
            for _ in range(n):
                if fillers:
                    fillers.popleft()()

        def attention(hp, ib, npump, defer_norm=False):
            qt = qkT[(hp, 0)]
            kt = qkT[(hp, 1)]
            po = {}
            for s in range(2):  # head 2*hp + s
                po[s] = ps_acc.tile(
                    [65, 512], F32, name=f"ps_ot_{hp}_{ib}_{s}", tag="ot", bufs=2
                )
            njc = 4 * (ib + 1)
            for jc in range(njc):
                r = jc - 4 * ib
                col0 = max(r, 0) * 128
                pst = ps_work.tile([128, 1024], F32, name=f"ps_st_{hp}_{ib}_{jc}", tag="ps")
                for s in range(2):
                    r0 = 64 * s
                    nc.tensor.matmul(
                        pst[:, 512 * s + col0 : 512 * s + 512],
                        kt[r0 : r0 + 64, jc * 128 : (jc + 1) * 128],
                        qt[r0 : r0 + 64, ib * 512 + col0 : (ib + 1) * 512],
                        start=True,
                        stop=True,
                    )
                est = est_pool.tile([128, 1024], BF16, name=f"est_{hp}_{ib}_{jc}", tag="est")
                nc.scalar.activation(
                    est.rearrange("p (a f) -> p a f", a=2)[:, :, col0:512],
                    pst.rearrange("p (a f) -> p a f", a=2)[:, :, col0:512],
                    AF.Exp,
                    scale=SCALE,
                )
                if r >= 0:
                    for s in range(2):
                        nc.vector.tensor_tensor(
                            est[:, 512 * s + col0 : 512 * s + col0 + 128],
                            est[:, 512 * s + col0 : 512 * s + col0 + 128],
                            tri[:],
                            op=ALU.mult,
                        )
                pump(npump)
                for s in range(2):
                    h = 2 * hp + s
                    nc.tensor.matmul(
                        po[s][:, col0:512],
                        Vp[jc][:, h * 65 : h * 65 + 65],
                        est[:, 512 * s + col0 : 512 * s + 512],
                        start=(jc == 0),
                        stop=(jc == njc - 1),
                    )
                pump(1)
            # normalization: otu copies inline (frees PSUM); the rest of
            # the chain (scatter -> recip -> gather -> bcast -> mult) is
            # cross-queue latency, so it becomes filler units pumped into
            # the NEXT attention's jc loop (defer_norm) unless last.
            otu = {}
            for s in range(2):
                otu[s] = nrm_pool.tile(
                    [65, 512], F32, name=f"otu_{hp}_{ib}_{s}", tag="otu", bufs=4
                )
                nc.vector.tensor_copy(otu[s][:], po[s][:, :])

            def mk_units():
                zs, zr, zinv, zb = {}, {}, {}, {}
                for s in range(2):
                    zs[s] = nrm_pool.tile([128, 4], F32, name=f"zs_{hp}_{ib}_{s}", tag="zs")
                    zr[s] = nrm_pool.tile([128, 4], F32, name=f"zr_{hp}_{ib}_{s}", tag="zr")
                    zinv[s] = nrm_pool.tile([1, 512], F32, name=f"zinv_{hp}_{ib}_{s}", tag="zinv")
                    zb[s] = nrm_pool.tile([64, 512], F32, name=f"zb_{hp}_{ib}_{s}", tag="zb")

                def u1():
                    for s in range(2):
                        nc.gpsimd.dma_start(zs[s][:], otu[s][64:65, :])

                def u2():
                    for s in range(2):
                        nc.vector.reciprocal(zr[s][:], zs[s][:])

                def u3():
                    for s in range(2):
                        nc.gpsimd.dma_start(zinv[s][:], zr[s][:])
                        nc.gpsimd.partition_broadcast(zb[s][:], zinv[s][:])

                def u4():
                    for s in range(2):
                        nc.vector.tensor_tensor(
                            ATn[hp][64 * s : 64 * s + 64, ib * 512 : (ib + 1) * 512],
                            otu[s][0:64, :],
                            zb[s][:],
                            op=ALU.mult,
                        )

                return [u1, u2, u3, u4]

            units = mk_units()
            if defer_norm:
                fillers.extendleft(reversed(units))
            else:
                for u in units:
                    u()

        def proj_units(tp):
            y_sb = y_pool.tile([128, C], F32, name=f"y_sb_{tp}", tag="y_sb")
            units = []
            for oc in range(2):
                py = ps_acc.tile([128, 384], F32, name=f"ps_y_{tp}_{oc}", tag="acc")

                def u1(py=py, oc=oc):
                    for cp in range(3):
                        nc.tensor.matmul(
                            py[:],
                            ATn[cp][:, tp * 128 : (tp + 1) * 128],
                            Wp[cp][:, oc * 384 : (oc + 1) * 384],
                            start=(cp == 0),
                            stop=False,
                        )

                def u2(py=py, oc=oc, last=(oc == 1)):
                    for cp in range(3, CC):
                        nc.tensor.matmul(
                            py[:],
                            ATn[cp][:, tp * 128 : (tp + 1) * 128],
                            Wp[cp][:, oc * 384 : (oc + 1) * 384],
                            start=False,
                            stop=False,
                        )
                    nc.tensor.matmul(
                        py[:],
                        ones_row[:, 0:128],
                        bp_sb[:, oc * 384 : (oc + 1) * 384],
                        start=False,
                        stop=True,
                    )
                    nc.vector.tensor_copy(y_sb[:, oc * 384 : (oc + 1) * 384], py[:])
                    if last:
                        nc.sync.dma_start(y_d[tp * 128 : (tp + 1) * 128, :], y_sb[:])

                units += [u1, u2]
            return units

        # ---- emission schedule -----------------------------------------
        # Prologue: first two pairs + first four Vp chunks inline (these
        # cover the input DMA); the rest become fillers pumped into the
        # attention loop.
        for tb in range(TB):
            for u in qk_chain_units(0, 0, tb) + qk_chain_units(0, 1, tb):
                u()
        for tb in range(TB):
            for u in qk_chain_units(1, 0, tb) + qk_chain_units(1, 1, tb):
                u()
        for tp in range(2):
            for vc in range(2):
                for u in v_chain_units(tp, vc):
                    u()

        for vc in range(2):
            fillers.extend(v_chain_units(2, vc))
            fillers.extend(v_chain_units(3, vc))
        for hp in range(2, NP):
            for qk in range(2):
                for tb in range(TB):
                    fillers.extend(qk_chain_units(hp, qk, tb))
            # interleave remaining v chunks (tp 4..7) between pair chains
            for vc in range(2):
                fillers.extend(v_chain_units(2 + hp, vc))

        attention(0, 0, npump=2)
        attention(1, 0, npump=2)
        attention(2, 0, npump=2)
        attention(3, 0, npump=2)
        attention(4, 0, npump=2)
        attention(5, 0, npump=2)
        for tp in range(4):
            fillers.extend(proj_units(tp))
        attention(0, 1, npump=1)
        attention(1, 1, npump=1)
        attention(2, 1, npump=1)
        attention(3, 1, npump=1)
        attention(4, 1, npump=1)
        attention(5, 1, npump=1, defer_norm=False)
        pump(len(fillers))
        for tp in range(4, 8):
            for u in proj_units(tp):
                u()


def kernel(x, W_attn, b_attn, W_proj, b_proj, _trace=False, _trace_kwargs=None):
    import ml_dtypes

    bf16 = ml_dtypes.bfloat16

    x = np.asarray(x, np.float32)
    W_attn = np.asarray(W_attn, np.float32)
    b_attn = np.asarray(b_attn, np.float32)
    W_proj = np.ascontiguousarray(np.asarray(W_proj).astype(bf16))
    b_proj = np.ascontiguousarray(np.asarray(b_proj).astype(bf16)).reshape(1, C)

    # W_qk pair-major: col block hp*256 holds [q cols of pair hp | k cols]
    wqk = np.zeros((C, 2 * C), np.float32)
    bqk = np.zeros((128, 12), np.float32)
    for hp in range(NP):
        for qk in range(2):
            src = qk * C + hp * 128
            wqk[:, hp * 256 + qk * 128 : hp * 256 + qk * 128 + 128] = W_attn[
                :, src : src + 128
            ]
            bqk[:, hp * 2 + qk] = b_attn[src : src + 128]
    wqk = np.ascontiguousarray(wqk.astype(bf16))
    bqk = np.ascontiguousarray(bqk)
    wv = np.ascontiguousarray(W_attn[:, 2 * C :].astype(bf16))
    bv = np.ascontiguousarray(b_attn[2 * C :].astype(bf16)).reshape(1, C)

    if "prog" not in _PROGRAM_CACHE:
        _PROGRAM_CACHE["prog"] = build_program()
    nc = _PROGRAM_CACHE["prog"]

    in_maps = []
    for b in range(NCORES):
        in_maps.append(
            {
                "xT": np.ascontiguousarray(x[b].T.astype(bf16)),
                "wqk": wqk,
                "Wv": wv,
                "W_proj": W_proj,
                "bqk": bqk,
                "bv": bv,
                "b_proj": b_proj,
            }
        )
    res = run_bass_kernel_spmd(
        nc,
        in_maps,
        core_ids=list(range(NCORES)),
        trace=_trace,
        **(_trace_kwargs or {}),
    )
    out = np.stack([res.results[b]["y"] for b in range(NCORES)], axis=0)
    if _trace:
        return out, res
    return out


if __name__ == "__main__":
    rng = np.random.default_rng(0)
    x = rng.standard_normal((NCORES, T, C)).astype(np.float32)
    W_attn = (rng.standard_normal((C, 3 * C)) * 0.02).astype(np.float32)
    b_attn = np.zeros(3 * C, np.float32)
    W_proj = (rng.standard_normal((C, C)) * 0.02).astype(np.float32)
    b_proj = np.zeros(C, np.float32)
    y = kernel(x=x, W_attn=W_attn, b_attn=b_attn, W_proj=W_proj, b_proj=b_proj)
    print("out", y.shape, y.dtype, np.abs(y).max())

